# revision 13
# baseline (speedup 1.0000x reference)
"""Trainium2 Bass kernel for DiffusionConvolution (N=4096, F=16, K=3).

Reference computation:
    M = sum_k theta[k,0]*Wp[k] + theta[k,1]*WTp[k]        # [N, N]
    Y = X + M @ X

We never materialize M:
    Y = X + sum_t A_t @ (theta_t * X)   over the 2K term matrices.

Wp[0] and WTp[0] are identity matrices by construction (k=0 diffusion
power), so their terms reduce to (theta[0,0]+theta[0,1])*X and are folded
into the final X add — verified exactly at runtime with a fallback to the
general path. That cuts streamed W data by 1/3 and makes the dominant
identity contribution exact (the f32r matmul rounding only touches the
small diffusion terms; overall rel err ~5e-6).

Sharding: core c owns output rows [c*512, (c+1)*512). The TensorE
contracts over the partition dim, so each core gets the [4096, 512]
column slice of each remaining A_t.T, packed host-side into 32
DMA-friendly ~1.06MB slabs (one per 128-row contraction chunk). A slab
is nt per-term segments [theta_t*X head [128,16] | A_t.T body
[128,512]], so stationary operands travel with their data and any
term-prefix of a slab is contiguous — the last slab is sent as two
halves so the final PE drain is 2 matmuls, not 4. Each matmul:
stationary = head [128,16], moving = body [128,512] in float32r
(TF32-like, 1 cycle/row), all nt*32 accumulating into one [16,512]
PSUM bank; a final DVE add applies xscale*X. Output is Y.T per core;
host transposes + concatenates. No collectives.

Raw Bass (no TileContext): a linear pipeline on explicit semaphores.
The 4-byte fused-LDW matmul supports only ONE sync wait, and later DMA
completions on a shared semaphore can satisfy an earlier wait (16 SDMA
engines increment independently), so each slab slot gets its own
semaphore with at most one DMA in flight per sem — race-free by
construction. Per-core traffic ~34MB -> dense gapless stream at the
~25GB/s-per-SDMA-engine HBM rate (~85us); PE (~55us HAM-throttled)
hides under DMA. Measured ~100us end-to-end incl ~9us NEFF preamble.
"""

import numpy as np

N = 4096
F = 16
K = 3
NCORES = 8
ROWS = N // NCORES            # 512 output rows per core
PART = 128                    # partition dim / contraction tile
MC = N // PART                # 32 contraction chunks
NBUF = 12                     # slab buffering depth

MOVING_DTYPE = "float32r"     # "float32" for exact (4x slower PE)


def _install_ntff_shim():
    """The image's antenv lacks axon_hooks; register the ctypes NTFF hook so
    run_bass_kernel_spmd(trace=True) works. Harmless no-op on failure."""
    import sys
    import types

    if "antenv.axon_hooks" in sys.modules:
        return
    try:
        from trn_agent_boot.trn_boot import _ntff_profile_via_ctypes

        hook = _ntff_profile_via_ctypes("/opt/axon/libaxon_pjrt.so")
        mod = types.ModuleType("antenv.axon_hooks")
        mod._hook = hook
        mod.get_axon_ntff_profile_hook = lambda: mod._hook
        mod.set_axon_ntff_profile_hook = lambda h: setattr(mod, "_hook", h)
        sys.modules["antenv.axon_hooks"] = mod
        try:
            import antenv

            antenv.axon_hooks = mod
        except Exception:
            pass
    except Exception:
        pass


_NC_CACHE = {}


def _build_bass(nt):
    """Bass graph for nt term matrices.

    Slab = nt segments of [F head | ROWS body] (term-major), 4*nt*(F+ROWS)
    bytes per partition. Last slab split into two half-DMAs.
    """
    if nt in _NC_CACHE:
        return _NC_CACHE[nt]
    import contextlib

    import concourse.bass as bass  # noqa: F401
    import concourse.mybir as mybir

    f32 = mybir.dt.float32
    sb_dt = getattr(mybir.dt, MOVING_DTYPE)
    seg = F + ROWS               # one term's [head | body]
    wslab = nt * seg
    ntA = nt // 2                # terms in the first half of the last slab
    LAST = MC - 1

    nc = bass.Bass(
        trn_type="TRN2",
        target_bir_lowering=False,
        debug=False,
        num_devices=NCORES,
    )
    wp = nc.dram_tensor("wpack", [MC, PART, wslab], f32, kind="ExternalInput")
    xtd = nc.dram_tensor("xt", [F, ROWS], f32, kind="ExternalInput")
    outd = nc.dram_tensor("out", [F, ROWS], f32, kind="ExternalOutput")

    with (
        nc.semaphore("in_sem") as in_sem,
        nc.semaphore("pe_sem") as pe_sem,
        nc.semaphore("dve_sem") as dve_sem,
        nc.semaphore("out_sem") as out_sem,
        nc.semaphore("lastA_sem") as lastA_sem,
        nc.semaphore("lastB_sem") as lastB_sem,
        nc.sbuf_tensor("xts", [F, ROWS], f32) as xts,
        nc.sbuf_tensor("wsl", [PART, NBUF * wslab], sb_dt) as wsl,
        nc.sbuf_tensor("osb", [F, ROWS], f32) as osb,
        nc.psum_tensor("acc", [F, ROWS], f32) as acc,
        contextlib.ExitStack() as st,
    ):
        slot_sems = [
            st.enter_context(nc.semaphore(f"slot_sem{i}")) for i in range(NBUF)
        ]

        with nc.Block() as block:

            @block.sync
            def _(sync):
                sync.dma_start(xts[:], xtd[:]).then_inc(in_sem, 16)
                for mc in range(MC):
                    if mc >= NBUF:
                        # WAR: don't overwrite a slot PE hasn't consumed
                        sync.wait_ge(pe_sem, mc - NBUF + 1)
                    slot = (mc % NBUF) * wslab
                    if mc == LAST:
                        cut = ntA * seg
                        sync.dma_start(
                            wsl[:, slot : slot + cut],
                            wp[mc][:, :cut].bitcast(sb_dt),
                        ).then_inc(lastA_sem, 16)
                        sync.dma_start(
                            wsl[:, slot + cut : slot + wslab],
                            wp[mc][:, cut:].bitcast(sb_dt),
                        ).then_inc(lastB_sem, 16)
                    else:
                        sync.dma_start(
                            wsl[:, slot : slot + wslab], wp[mc].bitcast(sb_dt)
                        ).then_inc(slot_sems[mc % NBUF], 16)
                sync.wait_ge(out_sem, 16)

            @block.tensor
            def _(tensor):
                for mc in range(MC):
                    slot = (mc % NBUF) * wslab
                    if mc == LAST:
                        tensor.wait_ge(lastA_sem, 16)
                    else:
                        tensor.wait_ge(slot_sems[mc % NBUF], 16 * (mc // NBUF + 1))
                    for t in range(nt):
                        if mc == LAST and t == ntA:
                            tensor.wait_ge(lastB_sem, 16)
                        base = slot + t * seg
                        mm = tensor.matmul(
                            acc[:],
                            lhsT=wsl[:, base : base + F],
                            rhs=wsl[:, base + F : base + seg],
                            start=(mc == 0 and t == 0),
                            stop=(mc == MC - 1 and t == nt - 1),
                        )
                    mm.then_inc(pe_sem, 1)

            @block.vector
            def _(vector):
                vector.wait_ge(pe_sem, MC)
                vector.wait_ge(in_sem, 16)  # xt
                vector.tensor_add(osb[:], acc[:], xts[:]).then_inc(dve_sem, 1)

            @block.scalar
            def _(scalar):
                # output DMA on the Activation HWDGE ring — off the busy
                # sync ring, descriptors prepped in parallel
                scalar.wait_ge(dve_sem, 1)
                scalar.dma_start(outd[:], osb[:]).then_inc(out_sem, 16)

    _NC_CACHE[nt] = nc
    return nc


def _is_identity(A):
    """Exact check: A == eye(N), without materializing eye."""
    if np.count_nonzero(A) != N:
        return False
    return bool((np.diagonal(A) == 1.0).all())


def _pack_inputs(X, theta, Wp, WTp):
    X = np.ascontiguousarray(X, dtype=np.float32)
    theta = np.asarray(theta, dtype=np.float32)
    Wp = np.asarray(Wp, dtype=np.float32)
    WTp = np.asarray(WTp, dtype=np.float32)

    # Identity terms contribute theta*X directly; fold into the X add.
    terms = []       # (scale, matrix) for non-identity terms
    xscale = 1.0     # Y = X + ... -> the "1"
    for k in range(K):
        for j, A in ((0, Wp[k]), (1, WTp[k])):
            th = float(theta[k, j])
            if k == 0 and _is_identity(A):
                xscale += th
            else:
                terms.append((th, A))
    nt = len(terms)

    seg = F + ROWS
    Xr = X.reshape(MC, PART, F)

    # Slab mc, term t segment: [head | body]
    #   head[p, f] = th_t * X[mc*PART + p, f]
    #   body[p, n] = A_t[c*ROWS + n, mc*PART + p]
    pk = np.empty((NCORES, MC, PART, nt, seg), dtype=np.float32)
    head = pk[:, :, :, :, :F]
    body = pk[:, :, :, :, F:]
    hx = np.stack([th * Xr for th, _ in terms], axis=2)  # [MC, PART, nt, F]
    head[:] = hx[None]
    for t, (th, A) in enumerate(terms):
        v = A.T.reshape(MC, PART, NCORES, ROWS)  # strided view, no copy
        body[:, :, :, t, :] = v.transpose(2, 0, 1, 3)
    pk = pk.reshape(NCORES, MC, PART, nt * seg)

    in_maps = []
    for c in range(NCORES):
        in_maps.append(
            {
                "wpack": pk[c],
                "xt": np.ascontiguousarray(
                    (xscale * X[c * ROWS : (c + 1) * ROWS]).T
                ),
            }
        )
    return in_maps, nt


def run(inputs, trace=False, trace_kwargs=None):
    """Returns (Y [N, F] float32, BassKernelResults)."""
    _install_ntff_shim()
    from concourse.bass_utils import run_bass_kernel_spmd

    in_maps, nt = _pack_inputs(**inputs)
    nc = _build_bass(nt)
    res = run_bass_kernel_spmd(
        nc,
        in_maps,
        core_ids=list(range(NCORES)),
        trace=trace,
        **(trace_kwargs or {}),
    )
    outs = [np.asarray(r["out"]) for r in res.results]
    Y = np.concatenate([o.T for o in outs], axis=0)
    return np.ascontiguousarray(Y, dtype=np.float32), res


def kernel(**inputs):
    Y, _ = run(inputs, trace=False)
    return Y
